# revision 35
# baseline (speedup 1.0000x reference)
import os
import sys

sys.path.insert(0, "/opt/trn_rl_repo")

import numpy as np

S = 64
B = 262144
NCORES = 8
BC = B // NCORES  # 32768 batch per core
P = 128
F = BC // P  # 256 free elems per partition
TB = 2  # steps per block
NBLK = S // TB
FB = TB * F  # free width of a block tile

S_MIN = 0.01
S_MAX = 36500.0
E_LO = 1.3
E_HI = 5.5

_cache = {}


def _build(w):
    """Build the Bass program for one core given the runtime w values."""
    import concourse.bass as bass
    import concourse.bacc as bacc
    import concourse.mybir as mybir
    from concourse.tile import TileContext

    fp32 = mybir.dt.float32
    Op = mybir.AluOpType
    Act = mybir.ActivationFunctionType

    w0, w1, w2, w3, w4, w5, w6 = [float(x) for x in w]

    # alpha (early-branch slope) per rating r=1..4
    a1 = w5
    a2 = w4 * w6 / 2.0
    a3 = (w3 + 1.0) / 2.0 * w6
    a4 = w6
    # alpha(r) = min(p1*r+q1, p2*r+q2), exact at r=1..4
    p1 = a2 - a1
    q1 = 2.0 * a1 - a2
    p2 = a4 - a3
    q2 = 4.0 * a3 - 3.0 * a4
    assert p1 * 3 + q1 >= a3 - 1e-6 and p1 * 4 + q1 >= a4 - 1e-6, "alpha fit invalid"
    assert p2 * 1 + q2 >= a1 - 1e-6 and p2 * 2 + q2 >= a2 - 1e-6, "alpha fit invalid"
    assert w5 == 0.0, "kernel assumes w5 == 0 (rating-1 update collapses to 0)"
    # c_delta(r) = [0, 0, c3, c4] via  c3*relu(r-2) - (2*c3-c4)*relu(r-3)
    c3 = w3 * w6
    c4 = w6 / 2.0
    q3 = 2.0 * c3 - c4
    assert c3 >= 0.0 and q3 >= 0.0, "c_delta relu decomposition invalid"
    # c_gamma(r) = [0, cg2, 0, cg4];  gamma = e_hat * c_gamma
    cg2 = w4 * w6 / 2.0
    cg4 = w6 / 2.0
    e0c = float(np.clip(np.float32(w2), E_LO, E_HI))
    assert w0 > -90.0 and w1 > -90.0, "init ivl would need extra S_MIN clamp"

    nc = bacc.Bacc("TRN2", num_devices=NCORES)
    inp = nc.declare_dram_parameter("inp", [NBLK, P, FB], fp32, isOutput=False)
    rinp = nc.declare_dram_parameter("ru", [NBLK, P, FB], mybir.dt.uint8, isOutput=False)
    m2inp = nc.declare_dram_parameter("m2", [NBLK, P, FB], mybir.dt.uint8, isOutput=False)
    out = nc.declare_dram_parameter("out", [2, NBLK, P, FB], fp32, isOutput=True)

    with TileContext(nc) as tc:
        with (
            tc.tile_pool(name="pin", bufs=2) as pin,
            tc.tile_pool(name="pstate", bufs=3) as pstate,
            tc.tile_pool(name="pcoef", bufs=3) as pcoef,
            tc.tile_pool(name="pscr", bufs=2) as pscr,
            tc.tile_pool(name="pconst", bufs=1) as pconst,
            tc.tile_pool(name="psmall", bufs=2) as psmall,
        ):
            two_blk = pconst.tile([P, FB], fp32, tag="two")
            nc.vector.memset(two_blk[:], 2.0)

            def bias_ap(val, _seen={}):
                if val not in _seen:
                    t = pconst.tile([P, 1], fp32, tag=f"b{len(_seen)}")
                    nc.vector.memset(t[:], val)
                    _seen[val] = t
                return _seen[val][:]

            prev_i = None  # [P,F] AP slice of last step's ivl
            prev_e = None

            for k in range(NBLK):
                t0 = k * TB
                # ---- input block DMAs ----
                dt_blk = pin.tile([P, FB], fp32, tag="dt")
                r_blk = pin.tile([P, FB], mybir.dt.uint8, tag="r")
                m2_blk = pin.tile([P, FB], mybir.dt.uint8, tag="m2")
                nc.sync.dma_start(dt_blk[:], inp[k])
                nc.sync.dma_start(r_blk[:], rinp[k])
                nc.sync.dma_start(m2_blk[:], m2inp[k])

                i9 = pstate.tile([P, FB], fp32, tag="i9")
                e9 = pstate.tile([P, FB], fp32, tag="e9")

                # ---- ease scan for this block (sequential, DVE) ----
                dp_blk = pscr.tile([P, FB], fp32, tag="dp")
                # d' = relu(0.15*r - 0.25);  e_new = clamp(e_prev + (d' - 0.2))
                nc.scalar.activation(dp_blk[:], r_blk[:], Act.Relu, bias=bias_ap(-0.25), scale=0.15)
                for s in range(TB):
                    t = t0 + s
                    e_dst = e9[:, s * F : (s + 1) * F]
                    if t == 0:
                        nc.vector.memset(e_dst, e0c)
                    else:
                        e_src = prev_e if s == 0 else e9[:, (s - 1) * F : s * F]
                        nc.vector.scalar_tensor_tensor(
                            e_dst, e_src, -0.2, dp_blk[:, s * F : (s + 1) * F],
                            Op.add, Op.add,
                        )
                        nc.vector.tensor_scalar(e_dst, e_dst, E_LO, E_HI, Op.max, Op.min)

                # ---- e_hat block: e_prev per step, patched to 2.0 where r==2 ----
                eh_blk = pcoef.tile([P, FB], fp32, tag="eh")
                if k == 0:
                    nc.vector.memset(eh_blk[:, 0:F], 1.0)  # t=0 slot unused
                else:
                    nc.sync.dma_start(eh_blk[:, 0:F], prev_e)
                nc.sync.dma_start(eh_blk[:, F:FB], e9[:, 0 : (TB - 1) * F])
                nc.vector.copy_predicated(eh_blk[:], m2_blk[:], two_blk[:])

                # ---- coefficient blocks ----
                Z_blk = pcoef.tile([P, FB], fp32, tag="Z")
                nc.vector.tensor_tensor(Z_blk[:], dt_blk[:], eh_blk[:], Op.mult)

                cd_blk = pscr.tile([P, FB], fp32, tag="cd")
                rb_blk = pscr.tile([P, FB], fp32, tag="rb")
                nc.scalar.activation(cd_blk[:], r_blk[:], Act.Relu, bias=bias_ap(-2.0 * c3), scale=c3)
                nc.scalar.activation(rb_blk[:], r_blk[:], Act.Relu, bias=bias_ap(-3.0 * q3), scale=q3)
                nc.gpsimd.tensor_tensor(cd_blk[:], cd_blk[:], rb_blk[:], Op.subtract)
                del_blk = pcoef.tile([P, FB], fp32, tag="del")
                nc.vector.tensor_tensor(del_blk[:], Z_blk[:], cd_blk[:], Op.mult)

                m4_blk = pscr.tile([P, FB], fp32, tag="m4")
                nc.scalar.activation(m4_blk[:], r_blk[:], Act.Relu, bias=bias_ap(-3.0), scale=1.0)
                cg_blk = pscr.tile([P, FB], fp32, tag="cg")
                nc.scalar.activation(cg_blk[:], m2_blk[:], Act.Copy, bias=0.0, scale=cg2)
                m4h_blk = pscr.tile([P, FB], fp32, tag="m4h")
                nc.scalar.activation(m4h_blk[:], m4_blk[:], Act.Copy, bias=0.0, scale=cg4)
                nc.gpsimd.tensor_tensor(cg_blk[:], cg_blk[:], m4h_blk[:], Op.add)
                gam_blk = pcoef.tile([P, FB], fp32, tag="gam")
                nc.gpsimd.tensor_tensor(gam_blk[:], eh_blk[:], cg_blk[:], Op.mult)

                alp_blk = pcoef.tile([P, FB], fp32, tag="alp")
                t1_blk = pscr.tile([P, FB], fp32, tag="t2")
                nc.scalar.activation(t1_blk[:], r_blk[:], Act.Copy, bias=q1, scale=p1)
                rel_blk = pscr.tile([P, FB], fp32, tag="rel")
                nc.scalar.activation(rel_blk[:], r_blk[:], Act.Relu,
                                     bias=bias_ap(q1 - q2), scale=p1 - p2)
                nc.gpsimd.tensor_tensor(alp_blk[:], t1_blk[:], rel_blk[:], Op.subtract)

                # ---- sequential ivl chain ----
                for s in range(TB):
                    t = t0 + s
                    sl = slice(s * F, (s + 1) * F)
                    i_dst = i9[:, sl]
                    if t == 0:
                        m40 = psmall.tile([P, F], fp32, tag="m40")
                        nc.vector.tensor_scalar(m40[:], r_blk[:, sl], 4.0, None, Op.is_equal)
                        V = psmall.tile([P, F], fp32, tag="V")
                        nc.vector.tensor_scalar(V[:], m40[:], w1 - w0, w0, Op.mult, Op.add)
                    else:
                        i_prev = prev_i if s == 0 else i9[:, (s - 1) * F : s * F]
                        neg = psmall.tile([P, F], mybir.dt.uint8, tag="neg")
                        nc.vector.tensor_tensor(neg[:], i_prev, dt_blk[:, sl], Op.is_gt)
                        j = psmall.tile([P, F], fp32, tag="j")
                        nc.vector.tensor_tensor(j[:], i_prev, Z_blk[:, sl], Op.max)
                        E = psmall.tile([P, F], fp32, tag="E")
                        nc.vector.tensor_tensor(E[:], j[:], alp_blk[:, sl], Op.mult)
                        q = psmall.tile([P, F], fp32, tag="q")
                        nc.vector.tensor_tensor(q[:], i_prev, gam_blk[:, sl], Op.mult)
                        V = psmall.tile([P, F], fp32, tag="V")
                        nc.vector.tensor_tensor(V[:], q[:], del_blk[:, sl], Op.add)
                        nc.vector.copy_predicated(V[:], neg[:], E[:])
                    Wt = psmall.tile([P, F], fp32, tag="W")
                    nc.vector.tensor_scalar(Wt[:], V[:], 0.01, 0.99, Op.mult, Op.add)
                    nc.vector.scalar_tensor_tensor(i_dst, V[:], S_MAX, Wt[:], Op.min, Op.max)

                prev_i = i9[:, (TB - 1) * F : FB]
                prev_e = e9[:, (TB - 1) * F : FB]

                # ---- output DMAs ----
                nc.sync.dma_start(out[0, k], i9[:])
                nc.sync.dma_start(out[1, k], e9[:])

    if not nc.is_finalized():
        nc.finalize()
    return nc


def _install_ntff_hook():
    """Install the NTFF profiling hook that the agent image's antenv lacks."""
    import types

    try:
        from antenv.axon_hooks import get_axon_ntff_profile_hook  # noqa: F401

        return
    except ImportError:
        pass
    sys.path.insert(0, "/root/.axon_site")
    import antenv
    import trn_agent_boot.trn_boot as tb

    hook = tb._ntff_profile_via_ctypes("/opt/axon/libaxon_pjrt.so")
    mod = types.ModuleType("antenv.axon_hooks")
    mod._hook = hook
    mod.get_axon_ntff_profile_hook = lambda: mod._hook

    def set_axon_ntff_profile_hook(h):
        mod._hook = h

    mod.set_axon_ntff_profile_hook = set_axon_ntff_profile_hook
    sys.modules["antenv.axon_hooks"] = mod
    antenv.axon_hooks = mod


def kernel(inputs, w):
    from concourse.bass_utils import run_bass_kernel_spmd

    inputs = np.ascontiguousarray(inputs, dtype=np.float32)
    w = np.asarray(w, dtype=np.float32)

    key = w.tobytes()
    if key not in _cache:
        _cache[key] = _build(w)
    nc = _cache[key]

    # host-side shard + deinterleave: [S, B, 2] -> per core [2, S, P, F]
    in_maps = []
    for c in range(NCORES):
        shard = inputs[:, c * BC : (c + 1) * BC, :]  # [S, BC, 2]

        def blockify(v, dtype):
            return np.ascontiguousarray(
                v.reshape(NBLK, TB, P, F).transpose(0, 2, 1, 3).reshape(NBLK, P, TB * F)
            ).astype(dtype)

        dt_a = blockify(shard[:, :, 0], np.float32)
        r_a = blockify(shard[:, :, 1], np.uint8)
        m2_a = (r_a == 2).astype(np.uint8)
        in_maps.append({"inp": dt_a, "ru": r_a, "m2": m2_a})

    trace = bool(int(os.environ.get("BASS_KERNEL_TRACE", "0")))
    if trace:
        _install_ntff_hook()
    res = run_bass_kernel_spmd(nc, in_maps, core_ids=list(range(NCORES)), trace=trace)
    if trace:
        print(f"HW exec time: {res.exec_time_ns} ns", flush=True)
        kernel.last_exec_time_ns = res.exec_time_ns
        kernel.last_profile = res

    outputs = np.empty((S, B, 2), dtype=np.float32)
    for c in range(NCORES):
        o = res.results[c]["out"]  # [2, NBLK, P, TB*F]
        for pl in range(2):
            v = o[pl].reshape(NBLK, P, TB, F).transpose(0, 2, 1, 3)
            outputs[:, c * BC : (c + 1) * BC, pl] = v.reshape(S, BC)
    final_state = outputs[-1].copy()
    return outputs, final_state


# revision 36
# speedup vs baseline: 1.0058x; 1.0058x over previous
import os
import sys

sys.path.insert(0, "/opt/trn_rl_repo")

import numpy as np

S = 64
B = 262144
NCORES = 8
BC = B // NCORES  # 32768 batch per core
P = 128
F = BC // P  # 256 free elems per partition
TB = 2  # steps per block
NBLK = S // TB
FB = TB * F  # free width of a block tile

S_MIN = 0.01
S_MAX = 36500.0
E_LO = 1.3
E_HI = 5.5

_cache = {}


def _build(w):
    """Build the Bass program for one core given the runtime w values."""
    import concourse.bass as bass
    import concourse.bacc as bacc
    import concourse.mybir as mybir
    from concourse.tile import TileContext

    fp32 = mybir.dt.float32
    Op = mybir.AluOpType
    Act = mybir.ActivationFunctionType

    w0, w1, w2, w3, w4, w5, w6 = [float(x) for x in w]

    # alpha (early-branch slope) per rating r=1..4
    a1 = w5
    a2 = w4 * w6 / 2.0
    a3 = (w3 + 1.0) / 2.0 * w6
    a4 = w6
    # alpha(r) = min(p1*r+q1, p2*r+q2), exact at r=1..4
    p1 = a2 - a1
    q1 = 2.0 * a1 - a2
    p2 = a4 - a3
    q2 = 4.0 * a3 - 3.0 * a4
    assert p1 * 3 + q1 >= a3 - 1e-6 and p1 * 4 + q1 >= a4 - 1e-6, "alpha fit invalid"
    assert p2 * 1 + q2 >= a1 - 1e-6 and p2 * 2 + q2 >= a2 - 1e-6, "alpha fit invalid"
    assert w5 == 0.0, "kernel assumes w5 == 0 (rating-1 update collapses to 0)"
    # c_delta(r) = [0, 0, c3, c4] via  c3*relu(r-2) - (2*c3-c4)*relu(r-3)
    c3 = w3 * w6
    c4 = w6 / 2.0
    q3 = 2.0 * c3 - c4
    assert c3 >= 0.0 and q3 >= 0.0, "c_delta relu decomposition invalid"
    # c_gamma(r) = [0, cg2, 0, cg4];  gamma = e_hat * c_gamma
    cg2 = w4 * w6 / 2.0
    cg4 = w6 / 2.0
    e0c = float(np.clip(np.float32(w2), E_LO, E_HI))
    assert w0 > -90.0 and w1 > -90.0, "init ivl would need extra S_MIN clamp"

    nc = bacc.Bacc("TRN2", num_devices=NCORES)
    inp = nc.declare_dram_parameter("inp", [NBLK, P, FB], fp32, isOutput=False)
    rinp = nc.declare_dram_parameter("ru", [NBLK, P, FB], mybir.dt.uint8, isOutput=False)
    m2inp = nc.declare_dram_parameter("m2", [NBLK, P, FB], mybir.dt.uint8, isOutput=False)
    out = nc.declare_dram_parameter("out", [2, NBLK, P, FB], fp32, isOutput=True)

    with TileContext(nc) as tc:
        with (
            tc.tile_pool(name="pin", bufs=2) as pin,
            tc.tile_pool(name="pstate", bufs=3) as pstate,
            tc.tile_pool(name="pcoef", bufs=4) as pcoef,
            tc.tile_pool(name="pscr", bufs=3) as pscr,
            tc.tile_pool(name="pconst", bufs=1) as pconst,
            tc.tile_pool(name="psmall", bufs=2) as psmall,
        ):
            two_blk = pconst.tile([P, FB], fp32, tag="two")
            nc.vector.memset(two_blk[:], 2.0)

            def bias_ap(val, _seen={}):
                if val not in _seen:
                    t = pconst.tile([P, 1], fp32, tag=f"b{len(_seen)}")
                    nc.vector.memset(t[:], val)
                    _seen[val] = t
                return _seen[val][:]

            prev_i = None  # [P,F] AP slice of last step's ivl
            prev_e = None

            for k in range(NBLK):
                t0 = k * TB
                # ---- input block DMAs ----
                dt_blk = pin.tile([P, FB], fp32, tag="dt")
                r_blk = pin.tile([P, FB], mybir.dt.uint8, tag="r")
                m2_blk = pin.tile([P, FB], mybir.dt.uint8, tag="m2")
                nc.sync.dma_start(dt_blk[:], inp[k])
                nc.sync.dma_start(r_blk[:], rinp[k])
                nc.sync.dma_start(m2_blk[:], m2inp[k])

                i9 = pstate.tile([P, FB], fp32, tag="i9")
                e9 = pstate.tile([P, FB], fp32, tag="e9")

                # ---- ease scan for this block (sequential, DVE) ----
                dp_blk = pscr.tile([P, FB], fp32, tag="dp")
                # d' = relu(0.15*r - 0.25);  e_new = clamp(e_prev + (d' - 0.2))
                nc.scalar.activation(dp_blk[:], r_blk[:], Act.Relu, bias=bias_ap(-0.25), scale=0.15)
                for s in range(TB):
                    t = t0 + s
                    e_dst = e9[:, s * F : (s + 1) * F]
                    if t == 0:
                        nc.vector.memset(e_dst, e0c)
                    else:
                        e_src = prev_e if s == 0 else e9[:, (s - 1) * F : s * F]
                        nc.vector.scalar_tensor_tensor(
                            e_dst, e_src, -0.2, dp_blk[:, s * F : (s + 1) * F],
                            Op.add, Op.add,
                        )
                        nc.vector.tensor_scalar(e_dst, e_dst, E_LO, E_HI, Op.max, Op.min)

                # ---- e_hat block: e_prev per step, patched to 2.0 where r==2 ----
                eh_blk = pcoef.tile([P, FB], fp32, tag="eh")
                if k == 0:
                    nc.vector.memset(eh_blk[:, 0:F], 1.0)  # t=0 slot unused
                else:
                    nc.sync.dma_start(eh_blk[:, 0:F], prev_e)
                nc.sync.dma_start(eh_blk[:, F:FB], e9[:, 0 : (TB - 1) * F])
                nc.vector.copy_predicated(eh_blk[:], m2_blk[:], two_blk[:])

                # ---- coefficient blocks ----
                Z_blk = pcoef.tile([P, FB], fp32, tag="Z")
                nc.vector.tensor_tensor(Z_blk[:], dt_blk[:], eh_blk[:], Op.mult)

                cd_blk = pscr.tile([P, FB], fp32, tag="cd")
                rb_blk = pscr.tile([P, FB], fp32, tag="rb")
                nc.scalar.activation(cd_blk[:], r_blk[:], Act.Relu, bias=bias_ap(-2.0 * c3), scale=c3)
                nc.scalar.activation(rb_blk[:], r_blk[:], Act.Relu, bias=bias_ap(-3.0 * q3), scale=q3)
                nc.gpsimd.tensor_tensor(cd_blk[:], cd_blk[:], rb_blk[:], Op.subtract)
                del_blk = pcoef.tile([P, FB], fp32, tag="del")
                nc.vector.tensor_tensor(del_blk[:], Z_blk[:], cd_blk[:], Op.mult)

                m4_blk = pscr.tile([P, FB], fp32, tag="m4")
                nc.scalar.activation(m4_blk[:], r_blk[:], Act.Relu, bias=bias_ap(-3.0), scale=1.0)
                cg_blk = pscr.tile([P, FB], fp32, tag="cg")
                nc.scalar.activation(cg_blk[:], m2_blk[:], Act.Copy, bias=0.0, scale=cg2)
                m4h_blk = pscr.tile([P, FB], fp32, tag="m4h")
                nc.scalar.activation(m4h_blk[:], m4_blk[:], Act.Copy, bias=0.0, scale=cg4)
                nc.gpsimd.tensor_tensor(cg_blk[:], cg_blk[:], m4h_blk[:], Op.add)
                gam_blk = pcoef.tile([P, FB], fp32, tag="gam")
                nc.gpsimd.tensor_tensor(gam_blk[:], eh_blk[:], cg_blk[:], Op.mult)

                alp_blk = pcoef.tile([P, FB], fp32, tag="alp")
                t1_blk = pscr.tile([P, FB], fp32, tag="t2")
                nc.scalar.activation(t1_blk[:], r_blk[:], Act.Copy, bias=q1, scale=p1)
                rel_blk = pscr.tile([P, FB], fp32, tag="rel")
                nc.scalar.activation(rel_blk[:], r_blk[:], Act.Relu,
                                     bias=bias_ap(q1 - q2), scale=p1 - p2)
                nc.gpsimd.tensor_tensor(alp_blk[:], t1_blk[:], rel_blk[:], Op.subtract)

                # ---- sequential ivl chain ----
                for s in range(TB):
                    t = t0 + s
                    sl = slice(s * F, (s + 1) * F)
                    i_dst = i9[:, sl]
                    if t == 0:
                        m40 = psmall.tile([P, F], fp32, tag="m40")
                        nc.vector.tensor_scalar(m40[:], r_blk[:, sl], 4.0, None, Op.is_equal)
                        V = psmall.tile([P, F], fp32, tag="V")
                        nc.vector.tensor_scalar(V[:], m40[:], w1 - w0, w0, Op.mult, Op.add)
                    else:
                        i_prev = prev_i if s == 0 else i9[:, (s - 1) * F : s * F]
                        neg = psmall.tile([P, F], mybir.dt.uint8, tag="neg")
                        nc.vector.tensor_tensor(neg[:], i_prev, dt_blk[:, sl], Op.is_gt)
                        j = psmall.tile([P, F], fp32, tag="j")
                        nc.vector.tensor_tensor(j[:], i_prev, Z_blk[:, sl], Op.max)
                        E = psmall.tile([P, F], fp32, tag="E")
                        nc.vector.tensor_tensor(E[:], j[:], alp_blk[:, sl], Op.mult)
                        q = psmall.tile([P, F], fp32, tag="q")
                        nc.vector.tensor_tensor(q[:], i_prev, gam_blk[:, sl], Op.mult)
                        V = psmall.tile([P, F], fp32, tag="V")
                        nc.vector.tensor_tensor(V[:], q[:], del_blk[:, sl], Op.add)
                        nc.vector.copy_predicated(V[:], neg[:], E[:])
                    Wt = psmall.tile([P, F], fp32, tag="W")
                    nc.vector.tensor_scalar(Wt[:], V[:], 0.01, 0.99, Op.mult, Op.add)
                    nc.vector.scalar_tensor_tensor(i_dst, V[:], S_MAX, Wt[:], Op.min, Op.max)

                prev_i = i9[:, (TB - 1) * F : FB]
                prev_e = e9[:, (TB - 1) * F : FB]

                # ---- output DMAs ----
                nc.sync.dma_start(out[0, k], i9[:])
                nc.sync.dma_start(out[1, k], e9[:])

    if not nc.is_finalized():
        nc.finalize()
    return nc


def _install_ntff_hook():
    """Install the NTFF profiling hook that the agent image's antenv lacks."""
    import types

    try:
        from antenv.axon_hooks import get_axon_ntff_profile_hook  # noqa: F401

        return
    except ImportError:
        pass
    sys.path.insert(0, "/root/.axon_site")
    import antenv
    import trn_agent_boot.trn_boot as tb

    hook = tb._ntff_profile_via_ctypes("/opt/axon/libaxon_pjrt.so")
    mod = types.ModuleType("antenv.axon_hooks")
    mod._hook = hook
    mod.get_axon_ntff_profile_hook = lambda: mod._hook

    def set_axon_ntff_profile_hook(h):
        mod._hook = h

    mod.set_axon_ntff_profile_hook = set_axon_ntff_profile_hook
    sys.modules["antenv.axon_hooks"] = mod
    antenv.axon_hooks = mod


def kernel(inputs, w):
    from concourse.bass_utils import run_bass_kernel_spmd

    inputs = np.ascontiguousarray(inputs, dtype=np.float32)
    w = np.asarray(w, dtype=np.float32)

    key = w.tobytes()
    if key not in _cache:
        _cache[key] = _build(w)
    nc = _cache[key]

    # host-side shard + deinterleave: [S, B, 2] -> per core [2, S, P, F]
    in_maps = []
    for c in range(NCORES):
        shard = inputs[:, c * BC : (c + 1) * BC, :]  # [S, BC, 2]

        def blockify(v, dtype):
            return np.ascontiguousarray(
                v.reshape(NBLK, TB, P, F).transpose(0, 2, 1, 3).reshape(NBLK, P, TB * F)
            ).astype(dtype)

        dt_a = blockify(shard[:, :, 0], np.float32)
        r_a = blockify(shard[:, :, 1], np.uint8)
        m2_a = (r_a == 2).astype(np.uint8)
        in_maps.append({"inp": dt_a, "ru": r_a, "m2": m2_a})

    trace = bool(int(os.environ.get("BASS_KERNEL_TRACE", "0")))
    if trace:
        _install_ntff_hook()
    res = run_bass_kernel_spmd(nc, in_maps, core_ids=list(range(NCORES)), trace=trace)
    if trace:
        print(f"HW exec time: {res.exec_time_ns} ns", flush=True)
        kernel.last_exec_time_ns = res.exec_time_ns
        kernel.last_profile = res

    outputs = np.empty((S, B, 2), dtype=np.float32)
    for c in range(NCORES):
        o = res.results[c]["out"]  # [2, NBLK, P, TB*F]
        for pl in range(2):
            v = o[pl].reshape(NBLK, P, TB, F).transpose(0, 2, 1, 3)
            outputs[:, c * BC : (c + 1) * BC, pl] = v.reshape(S, BC)
    final_state = outputs[-1].copy()
    return outputs, final_state
